# revision 3
# baseline (speedup 1.0000x reference)
# Trainium2 Bass kernel for nn_AutoregressiveCDE (8-core data parallel).
#
# Per batch row: Neural-CDE with piecewise-linear control. Initial controlled
# solve over knots [1, cu), then autoregressive extrapolate-and-solve for
# positions [cu, tu). 4 RK4 substeps per interval, vector field
#   dy/dt = F(y) @ c,
#   F(y) = tanh(wf @ softplus(w1 @ softplus(w0 @ y + b0) + b1) + bf).reshape(3,3)
#
# Mapping: batch 256 -> 32 columns per core (free dim); hidden width 128 on
# partitions. The whole scan is one serial chain of (tu-1)*16 MLP evals.
# softplus(x) = ln(1+exp(x)), tanh(x) = 1 - 2*exp(-ln(1+exp(2x))): every
# activation lives in the single natural_log_exp_and_others ACT table set.
#
# All cross-partition movement is done by tiny static-weight matmuls
# (replication / selector matrices); per-position history (y1, ys) is stored
# along the free dim of partition-0 tiles, so every engine AP starts at
# partition 0 (TRN2 requires partition-base-aligned engine access).
#
# Per-stage restructuring (gamma = RK4 stage step):
#   z1(y + gamma*k) = w0 @ y + gamma*w0rs*csum + w0ext @ q
#   q = (-2*gamma) * w .* cexp9,  w = exp(-ln(1+exp(2*zf + 2*bf)))
#   w0ext = w0 @ R (R: 3x replication),  w0rs = row sums of w0,
#   csum = c0+c1+c2, cexp9[p] = c[p mod 3].
# Critical path per eval: stt(q) -> mm_q -> exp -> ln -> mm2 -> exp -> ln ->
# mm3 -> exp -> ln -> exp -> stt(q').

import os
import numpy as np

T = 100
B = 256
H = 3
W = 128
N_SUB = 4
NCORES = 8
N = B // NCORES  # batch columns per core

f32 = np.float32
_CACHE = {}


def _build_program(tsf, tscan, cu, tu):
    """Build + compile the Bacc program. tsf/tscan: float32[T] host arrays."""
    import concourse.tile as tile
    from concourse import bacc, mybir

    AF = mybir.ActivationFunctionType
    ALU = mybir.AluOpType
    DT = mybir.dt.float32
    HH = H * H

    nc = bacc.Bacc(
        "TRN2",
        target_bir_lowering=False,
        debug=False,
        enable_asserts=False,
        num_devices=NCORES,
    )

    def din(name, shape):
        return nc.dram_tensor(name, shape, DT, kind="ExternalInput").ap()

    d_ysrow = din("ysrow", [1, T * N])
    d_w0T = din("w0T", [H, W])
    d_w0extT = din("w0extT", [HH, W])
    d_w0rsT = din("w0rsT", [1, W])
    d_w1T = din("w1T", [W, W])
    d_wfT = din("wfT", [W, HH])
    d_b0 = din("b0c", [W, 1])
    d_b1 = din("b1c", [W, 1])
    d_bf2 = din("bf2c", [HH, 1])
    d_selK1 = din("selK1", [1, 37])
    d_selK3 = din("selK3", [H, 4])
    d_selK9 = din("selK9", [HH, 4])
    d_out = nc.dram_tensor("outbuf", [1, T * N], DT, kind="ExternalOutput").ap()

    def F(x):
        return float(f32(x))

    # per-interval f32 scalars (mirroring the reference's f32 arithmetic)
    dt_k = [None] * tu
    h_k = [None] * tu
    ga_k = [None] * tu  # h/2
    gf_k = [None] * tu  # h/6
    c0_k = [None] * tu
    inv_dt = [None] * tu
    for k in range(1, tu):
        dt = f32(tscan[k] - tscan[k - 1])
        dt_k[k] = dt
        h = f32(dt / N_SUB)
        h_k[k] = h
        ga_k[k] = f32(f32(0.5) * h)
        gf_k[k] = f32(h / f32(6.0))
        c0_k[k] = f32(f32(tsf[k] - tsf[k - 1]) / dt)
        inv_dt[k] = f32(f32(1.0) / dt)

    with tile.TileContext(nc) as tc:
        with (
            tc.tile_pool(name="const", bufs=1) as constp,
            tc.tile_pool(name="big", bufs=3) as bigp,
            tc.tile_pool(name="small", bufs=3) as smallp,
            tc.tile_pool(name="qap", bufs=4) as qap,
            tc.tile_pool(name="ysbp", bufs=3) as ysbp,
            tc.tile_pool(name="cp", bufs=2) as cp,
            tc.tile_pool(name="pz1", bufs=2, space="PSUM") as pz1,
            tc.tile_pool(name="pz2", bufs=2, space="PSUM") as pz2,
            tc.tile_pool(name="psm", bufs=4, space="PSUM") as psm,
        ):
            # ---------------- constants ----------------
            t_w0T = constp.tile([H, W], DT)
            nc.sync.dma_start(t_w0T[:], d_w0T[:])
            t_w0extT = constp.tile([HH, W], DT)
            nc.sync.dma_start(t_w0extT[:], d_w0extT[:])
            t_w0rsT = constp.tile([1, W], DT)
            nc.sync.dma_start(t_w0rsT[:], d_w0rsT[:])
            t_w1T = constp.tile([W, W], DT)
            nc.sync.dma_start(t_w1T[:], d_w1T[:])
            t_wfT = constp.tile([W, HH], DT)
            nc.sync.dma_start(t_wfT[:], d_wfT[:])
            t_b0 = constp.tile([W, 1], DT)
            nc.sync.dma_start(t_b0[:], d_b0[:])
            t_b1 = constp.tile([W, 1], DT)
            nc.sync.dma_start(t_b1[:], d_b1[:])
            t_bf2 = constp.tile([HH, 1], DT)
            nc.sync.dma_start(t_bf2[:], d_bf2[:])
            t_selK1 = constp.tile([1, 37], DT)
            nc.sync.dma_start(t_selK1[:], d_selK1[:])
            t_selK3 = constp.tile([H, 4], DT)
            nc.sync.dma_start(t_selK3[:], d_selK3[:])
            t_selK9 = constp.tile([HH, 4], DT)
            nc.sync.dma_start(t_selK9[:], d_selK9[:])
            t_ysrow = constp.tile([1, T * N], DT)
            nc.sync.dma_start(t_ysrow[:], d_ysrow[:])

            # selector views
            sel_j0 = t_selK1[0:1, 0:9]     # [1,9]: 1 at p%3==0
            sel_j1 = t_selK1[0:1, 9:18]    # [1,9]: 1 at p%3==1
            sel_j2 = t_selK1[0:1, 18:27]   # [1,9]: 1 at p%3==2
            sel_ones13 = t_selK1[0:1, 27:30]  # [1,3] ones
            sel_one11 = t_selK1[0:1, 30:31]   # [1,1] one
            sel_y0a = t_selK1[0:1, 31:34]     # [1,3] (0, 1, ts0)
            sel_y0b = t_selK1[0:1, 34:37]     # [1,3] (ts0, 0, 0)
            sel_I3 = t_selK3[:, 0:3]          # [3,3] identity
            sel_I3c1 = t_selK3[:, 3:4]        # [3,1] (0,1,0)
            sel_R = t_selK9[:, 0:3]           # [9,3] rowsum3
            sel_Rc1 = t_selK9[:, 3:4]         # [9,1] rowsum3 col 1

            # persistent state
            y1buf = constp.tile([1, T * N], DT)

            def yrow(k):
                return t_ysrow[0:1, k * N:(k + 1) * N]

            def y1at(k):
                return y1buf[0:1, k * N:(k + 1) * N]

            # ---------------- init ----------------
            onesN = constp.tile([1, N], DT)
            nc.vector.memset(onesN[:], 1.0)
            # y1 at position 0 is the raw data channel
            nc.vector.tensor_copy(y1at(0), yrow(0))
            # positions >= tu are zero
            nc.vector.memset(y1buf[0:1, tu * N:T * N], 0.0)

            # initial ODE state y0 = (ts0, ys0, ts0*ys0)
            y0_ps = psm.tile([H, N], DT, tag="sm")
            nc.tensor.matmul(y0_ps[:], sel_y0a, yrow(0), start=True, stop=False)
            nc.tensor.matmul(y0_ps[:], sel_y0b, onesN[:], start=False, stop=True)
            ysb = ysbp.tile([H, N], DT, tag="ysb")
            nc.vector.tensor_copy(ysb[:], y0_ps[:])

            # ---------------- helper: one MLP eval's ACT chain ----------------
            def act_chain(z1):
                e1 = bigp.tile([W, N], DT, tag="e1")
                nc.scalar.activation(e1[:], z1[:], AF.Exp, bias=t_b0[:])
                h1 = bigp.tile([W, N], DT, tag="h1")
                nc.scalar.activation(h1[:], e1[:], AF.Ln, bias=1.0)
                z2 = pz2.tile([W, N], DT, tag="z2")
                nc.tensor.matmul(z2[:], t_w1T[:], h1[:])
                e2 = bigp.tile([W, N], DT, tag="e2")
                nc.scalar.activation(e2[:], z2[:], AF.Exp, bias=t_b1[:])
                h2 = bigp.tile([W, N], DT, tag="h2")
                nc.scalar.activation(h2[:], e2[:], AF.Ln, bias=1.0)
                zf = psm.tile([HH, N], DT, tag="sm")
                nc.tensor.matmul(zf[:], t_wfT[:], h2[:])
                u = smallp.tile([HH, N], DT, tag="u")
                nc.scalar.activation(u[:], zf[:], AF.Exp, bias=t_bf2[:], scale=2.0)
                v = smallp.tile([HH, N], DT, tag="v")
                nc.scalar.activation(v[:], u[:], AF.Ln, bias=1.0)
                wt = smallp.tile([HH, N], DT, tag="wt")
                nc.scalar.activation(wt[:], v[:], AF.Exp, scale=-1.0)
                return wt

            # ---------------- main loop ----------------
            # eval 0's z1
            z1 = pz1.tile([W, N], DT, tag="z1")
            nc.tensor.matmul(z1[:], t_w0T[:], ysb[:], start=True, stop=True)

            for k in range(1, tu):
                # ---- c prep for interval k (off critical path) ----
                c1t = cp.tile([1, N], DT, tag="c1t")
                c2t = cp.tile([1, N], DT, tag="c2t")
                c0t = cp.tile([1, N], DT, tag="c0t")
                tmp = cp.tile([1, N], DT, tag="ctmp")
                tmp2 = cp.tile([1, N], DT, tag="ctmp2")
                idt = F(inv_dt[k])
                if k < cu:
                    ya, yb = yrow(k), yrow(k - 1)  # data channel
                    b2 = F(f32(tsf[k - 1]) * inv_dt[k])
                    a2 = F(f32(tsf[k]) * inv_dt[k])
                else:
                    ya, yb = y1at(k - 1), y1at(k - 2)  # autoregressive
                    b2 = F(f32(tscan[k]) * inv_dt[k])
                    a2 = F((f32(2.0) * tscan[k] - tscan[k - 1]) * inv_dt[k])
                nc.vector.tensor_scalar_mul(tmp[:], yb[:], idt)
                nc.vector.scalar_tensor_tensor(
                    c1t[:], ya[:], idt, tmp[:], op0=ALU.mult, op1=ALU.subtract)
                nc.vector.tensor_scalar_mul(tmp2[:], yb[:], b2)
                nc.vector.scalar_tensor_tensor(
                    c2t[:], ya[:], a2, tmp2[:], op0=ALU.mult, op1=ALU.subtract)
                nc.vector.memset(c0t[:], F(c0_k[k]))

                csr = cp.tile([1, N], DT, tag="csr")
                nc.vector.tensor_add(csr[:], c1t[:], c2t[:])
                nc.vector.tensor_add(csr[:], csr[:], c0t[:])
                cs_a = cp.tile([1, N], DT, tag="cs_a")
                nc.vector.tensor_scalar_mul(cs_a[:], csr[:], F(ga_k[k]))
                cs_b = cp.tile([1, N], DT, tag="cs_b")
                nc.vector.tensor_scalar_mul(cs_b[:], csr[:], F(h_k[k]))
                cs_h = cs_b  # gamma_b == full-step sum == h

                cx_ps = psm.tile([HH, N], DT, tag="sm")
                nc.tensor.matmul(cx_ps[:], sel_j0, c0t[:], start=True, stop=False)
                nc.tensor.matmul(cx_ps[:], sel_j1, c1t[:], start=False, stop=False)
                nc.tensor.matmul(cx_ps[:], sel_j2, c2t[:], start=False, stop=True)
                cexp9 = cp.tile([HH, N], DT, tag="cexp9")
                nc.vector.tensor_copy(cexp9[:], cx_ps[:])

                ga, gb, gf, hh = F(ga_k[k]), F(h_k[k]), F(gf_k[k]), F(h_k[k])
                stage_gamma = [ga, ga, gb, gf]
                stage_cs = [cs_a, cs_a, cs_b]  # for z1 of stages 2,3,4

                for sub in range(N_SUB):
                    qa = None
                    for st in range(4):
                        wt = act_chain(z1)
                        q = smallp.tile([HH, N], DT, tag="q")
                        nc.vector.scalar_tensor_tensor(
                            q[:], wt[:], -2.0 * stage_gamma[st], cexp9[:],
                            op0=ALU.mult, op1=ALU.mult)
                        # qacc update
                        if st == 0:
                            qa = qap.tile([HH, N], DT, tag="qa")
                            nc.vector.tensor_scalar_mul(qa[:], q[:], 1.0 / 3.0)
                        elif st in (1, 2):
                            qan = qap.tile([HH, N], DT, tag="qa")
                            nc.vector.scalar_tensor_tensor(
                                qan[:], q[:], (2.0 / 3.0) if st == 1 else (1.0 / 3.0),
                                qa[:], op0=ALU.mult, op1=ALU.add)
                            qa = qan
                        else:
                            qan = qap.tile([HH, N], DT, tag="qa")
                            nc.vector.tensor_add(qan[:], q[:], qa[:])
                            qa = qan

                        last_eval = (k == tu - 1 and sub == N_SUB - 1 and st == 3)
                        if st < 3:
                            z1 = pz1.tile([W, N], DT, tag="z1")
                            nc.tensor.matmul(z1[:], t_w0T[:], ysb[:], start=True, stop=False)
                            nc.tensor.matmul(z1[:], t_w0rsT[:], stage_cs[st][:], start=False, stop=False)
                            nc.tensor.matmul(z1[:], t_w0extT[:], q[:], start=False, stop=True)
                        elif not last_eval:
                            # z1 for the next substep/interval's first eval
                            z1 = pz1.tile([W, N], DT, tag="z1")
                            nc.tensor.matmul(z1[:], t_w0T[:], ysb[:], start=True, stop=False)
                            nc.tensor.matmul(z1[:], t_w0rsT[:], cs_h[:], start=False, stop=False)
                            nc.tensor.matmul(z1[:], t_w0extT[:], qa[:], start=False, stop=True)

                    # end of substep: y_next (off critical path)
                    yn_ps = psm.tile([H, N], DT, tag="sm")
                    nc.tensor.matmul(yn_ps[:], sel_I3, ysb[:], start=True, stop=False)
                    nc.tensor.matmul(yn_ps[:], sel_ones13, cs_h[:], start=False, stop=False)
                    nc.tensor.matmul(yn_ps[:], sel_R, qa[:], start=False, stop=True)
                    if sub == N_SUB - 1:
                        # y1 at position k (row 1 of y_next)
                        y1_ps = psm.tile([1, N], DT, tag="sm")
                        nc.tensor.matmul(y1_ps[:], sel_I3c1, ysb[:], start=True, stop=False)
                        nc.tensor.matmul(y1_ps[:], sel_one11, cs_h[:], start=False, stop=False)
                        nc.tensor.matmul(y1_ps[:], sel_Rc1, qa[:], start=False, stop=True)
                        nc.vector.tensor_copy(y1at(k), y1_ps[:])
                    ysbn = ysbp.tile([H, N], DT, tag="ysb")
                    nc.vector.tensor_copy(ysbn[:], yn_ps[:])
                    ysb = ysbn

            nc.sync.dma_start(d_out[:], y1buf[:])

    nc.compile()
    return nc


def _host_prep(inputs):
    ts = np.asarray(inputs["ts"], f32)
    ys = np.asarray(inputs["ys"], f32)
    cu = int(inputs["control_until"])
    tu = int(inputs["train_until"])
    w0 = np.asarray(inputs["w0"], f32)
    b0 = np.asarray(inputs["b0"], f32)
    w1 = np.asarray(inputs["w1"], f32)
    b1 = np.asarray(inputs["b1"], f32)
    wf = np.asarray(inputs["wf"], f32)
    bf = np.asarray(inputs["bf"], f32)

    w0extT = np.zeros((H * H, W), f32)
    for i in range(H):
        for j in range(H):
            w0extT[i * H + j, :] = w0[:, i]
    ts0 = f32(ts[0])

    selK1 = np.zeros((1, 37), f32)
    for p in range(9):
        selK1[0, 0 + p] = 1.0 if p % 3 == 0 else 0.0
        selK1[0, 9 + p] = 1.0 if p % 3 == 1 else 0.0
        selK1[0, 18 + p] = 1.0 if p % 3 == 2 else 0.0
    selK1[0, 27:30] = 1.0
    selK1[0, 30] = 1.0
    selK1[0, 31:34] = [0.0, 1.0, ts0]
    selK1[0, 34:37] = [ts0, 0.0, 0.0]
    selK3 = np.zeros((H, 4), f32)
    selK3[:, 0:3] = np.eye(H, dtype=f32)
    selK3[1, 3] = 1.0
    selK9 = np.zeros((H * H, 4), f32)
    for i in range(H):
        for j in range(H):
            selK9[i * H + j, i] = 1.0
    selK9[3:6, 3] = selK9[3:6, 1]
    selK9[:, 3] = 0.0
    selK9[3:6, 3] = 1.0

    shared = dict(
        w0T=np.ascontiguousarray(w0.T),
        w0extT=w0extT,
        w0rsT=np.ascontiguousarray(w0.sum(axis=1, dtype=f32).reshape(1, W)),
        w1T=np.ascontiguousarray(w1.T),
        wfT=np.ascontiguousarray(wf.T),
        b0c=b0.reshape(W, 1),
        b1c=b1.reshape(W, 1),
        bf2c=(f32(2.0) * bf).reshape(H * H, 1),
        selK1=selK1,
        selK3=selK3,
        selK9=selK9,
    )
    in_maps = []
    for c in range(NCORES):
        ys_c = ys[c * N:(c + 1) * N, :]  # [N, T]
        ysrow = np.ascontiguousarray(ys_c.T.reshape(1, T * N))  # [1, T*N]
        m = dict(shared)
        m["ysrow"] = ysrow
        in_maps.append(m)
    return ts, cu, tu, in_maps


def kernel(**inputs):
    from concourse import bass_utils

    ts, cu, tu, in_maps = _host_prep(inputs)
    tscan = np.linspace(0.0, 1.0, T).astype(f32)

    key = (ts.tobytes(), cu, tu)
    if key not in _CACHE:
        _CACHE[key] = _build_program(ts, tscan, cu, tu)
    nc = _CACHE[key]

    res = bass_utils.run_bass_kernel_spmd(
        nc, in_maps, core_ids=list(range(NCORES)), trace=False)

    out = np.zeros((B, T), f32)
    for c in range(NCORES):
        ob = res.results[c]["outbuf"].reshape(T, N)  # [T, N]
        out[c * N:(c + 1) * N, :] = ob.T
    return out
